# revision 14
# baseline (speedup 1.0000x reference)
"""EntropyPoolLayer Trainium2 kernel.

Math: out[n,oh,ow,c] = x[n, 2oh+di, 2ow+dj, c] for the window element whose
global value-count is minimal (entropy -p log p is strictly increasing in p
for p < 1/e, and max p ~ 0.04 here, so argmin entropy == argmin count, with
identical tie structure; ties resolved to the first window index k = 2di+dj).

Plan (8 NeuronCores, batch-sharded 4/core, SBUF layout partition = h):
  Phase 1: per-core exact histogram of key = round(10x)+64 in [0,128) via
           PE joint counting of hi-thermometer x lo-onehot indicator planes.
  Host:    merge counts, replicate the reference's f32 entropy per bin,
           rank bins by entropy (equal ent -> equal rank), build the fp16
           score table S[b] = (nr - rank[b])*4 + 3 (higher = rarer).
  Phase 2 (fast path): per-element score via a 23-pass custom-DVE sweep in
           fp16 over bins [32, 97] only:
             pass 0  RANGEINIT: init acc, keys outside [32,98) get a shared
                     overflow score OVF > all table scores (tail keys are
                     rare and have strictly smaller counts than any in-range
                     key -- host-verified), bin 32 scored.
             pass j  LUT3: 3 consecutive bins per pass; values come from
                     C0/C1 scalar APs plus Latch(Src0) = a per-pass scratch
                     cell spliced in front of the key stream.
           comparable = S - dj - 2*di (dj via in-place -1 on odd-w columns,
           di via per-partition rbias); window argmax == lexicographic min
           (rank, k). Winner key recovered by select chains; value =
           (key-64)*0.1.
  Phase 2 (fallback, exact over all bins): the original 54-pass f32 LUT2
           sweep with packed comparable 65536 - (rank*512 + b), used if the
           host detects tail/in-range count overlap.
"""

import re
import sys

import numpy as np

sys.path.insert(0, "/opt/trn_rl_repo")

import concourse.bacc as bacc  # noqa: E402
import concourse.mybir as mybir  # noqa: E402
import concourse.tile as tile  # noqa: E402
from concourse.bass_utils import run_bass_kernel_spmd  # noqa: E402
from concourse.dve_ops import OPS, DveOp  # noqa: E402
from concourse.dve_spec import (  # noqa: E402
    C0,
    C1,
    C2,
    AluOp,
    Bin,
    One,
    Spec,
    Src0,
    Src1,
    Zero,
    eq,
    maxx,
    select,
)

F32 = mybir.dt.float32
I32 = mybir.dt.int32
I16 = mybir.dt.int16
F16 = mybir.dt.float16
ALU = mybir.AluOpType
AF = mybir.ActivationFunctionType

N_CORES = 8
N, H, W, C = 32, 128, 160, 64
NPC = N // N_CORES  # batches per core
OH, OW = H // 2, W // 2
NB = 128  # key bins
FREE = W * C  # 10240 free elements per partition per batch
HO = OW * C  # 5120 pooled cols per batch
BIG = 65536.0

# fast-path sweep configuration: bins [RLO, RHI] swept exactly; keys outside
# get the shared overflow score via the RANGEC init pass
# (|key - CMID| > CRAD -> OVF, else 0), then LUT2 passes of 2 bins each.
RLO, RHI = 32, 97
CMID = (RLO + RHI) / 2.0  # 64.5
CRAD = (RHI - RLO) / 2.0  # 32.5; |k-CMID| > CRAD <=> k < RLO or k > RHI
NP2 = (RHI - RLO + 1 + 1) // 2  # LUT2 passes (2 bins each)
NP = 1 + NP2  # total passes
P2BASE = [min(RLO + 2 * j, RHI - 1) for j in range(NP2)]
# vals tensor column layout
VC_S0 = 0  # NP cols: per-pass s0
VC_S1 = NP  # NP cols: per-pass s1
VC_RB = 2 * NP  # 1 col: rbias = 2*(p%2)
VCOLS = 2 * NP + 1

# legacy fallback sweep window
BIN_LO = 9
NSWEEP = 108

# --------------------------------------------------------------------------
# Custom DVE ops
# --------------------------------------------------------------------------


def _register(name: str, spec: Spec) -> DveOp:
    import concourse.dve_ops as dve_ops

    for op in OPS:
        if op.name == name:
            return op
    tmp = DveOp(name, spec, subdim=False, uops_sha={})
    OPS.append(tmp)
    idx = next(i for i, o in enumerate(OPS) if o.name == name)
    dve_ops._SUB_OPCODE_FOR_NAME[name] = dve_ops._CUSTOM_DVE_ROW_BASE + idx
    dve_ops.CUSTOM_DVE_SPECS[name] = spec
    shas = {}
    for ver in ("v3", "v4"):
        try:
            tmp.compile(ver)
        except ValueError as e:
            m = re.search(r'="([0-9a-f]+)"', str(e))
            if m is None:
                raise
            shas[ver] = m.group(1)
    final = DveOp(name, spec, subdim=False, uops_sha=shas)
    OPS[idx] = final
    return final


# ---- fallback op: acc' = max(acc, (k==b)*v0 + (k==b+1)*v1) ----------------


def _lut2_ref(in0, in1, s0, s1, imm2):
    k = in0.astype(np.float32)
    val = (k == np.float32(imm2)) * s0 + (k == np.float32(imm2 + 1.0)) * s1
    return np.maximum(in1.astype(np.float32), val)


LUT2 = _register(
    "ENTROPY_LUT2",
    Spec(
        body=maxx(Src1, eq(Src0, C2) * C0 + eq(Src0, C2 + One) * C1),
        reference=_lut2_ref,
    ),
)

# ---- fast op --------------------------------------------------------------
# RANGEC (acc init): acc = s1 (overflow score) where |k - imm2| > s0, else 0.
# Keys are integers so with imm2 = 64.5, s0 = 32.5 this tags exactly
# k < 32 or k > 97. No Src1 (pure init); stream latches are avoided -- the
# runtime ucode executor crashes on Latch(Src0/Src1) specs.


def _rangec_ref(in0, in1, s0, s1, imm2):
    k = in0.astype(np.float32)
    return np.where(np.abs(k - np.float32(imm2)) > s0, s1 * np.ones_like(k), 0.0)


_AD = Bin(AluOp.ABSOLUTE_DIFF, Src0, C2)
RANGEC = _register(
    "ENTROPY_RANGEC",
    Spec(
        body=select(_AD > C0, C1, Zero),
        reference=_rangec_ref,
    ),
)

# --------------------------------------------------------------------------
# Kernel builders
# --------------------------------------------------------------------------


def build_phase1():
    """Exact 128-bin histogram via PE joint counting.

    Interleaved indicator planes per element (bf16, {0,1}):
      hi[e*8+s]  = [1, key>=16, ..., key>=112][s]   (thermometer, s=0 is ones)
      lo[e*16+r] = (int(key) & 15) == r
    PE contracts pages of 128 elements, 8 pages per matmul:
      psum[(pg, r), (pg, s)] += lo . hi   accumulated over all pages.
    Host keeps pg-diagonal blocks and differences the thermometer:
      count[16*h + r] = C[h, r] - C[h+1, r].
    """
    BF16 = mybir.dt.bfloat16
    nc = bacc.Bacc("TRN2", target_bir_lowering=False, debug=False)
    x = nc.dram_tensor("x", [NPC, H, W, C], F32, kind="ExternalInput")
    counts = nc.dram_tensor("counts", [128, 64], F32, kind="ExternalOutput")
    keys = nc.dram_tensor("keys", [128, NPC * FREE], F16, kind="ExternalOutput")
    xv = x[:].rearrange("n h w c -> h n (w c)")  # [128, NPC, FREE]
    QC = FREE // 4  # 2560 cols per chunk
    PG = 8  # pages per matmul: lhsT = lo [128, 8*16], rhs = hi [128, 8*8]
    NBLK = QC // PG

    with tile.TileContext(nc) as tc:
        with (
            tc.tile_pool(name="xp", bufs=1) as xp,
            tc.tile_pool(name="kp", bufs=2) as kp,
            tc.tile_pool(name="ip", bufs=1) as ip,
            tc.tile_pool(name="hp", bufs=2) as hp,
            tc.tile_pool(name="lp", bufs=1) as lp,
            tc.tile_pool(name="ps", bufs=1, space="PSUM") as ps,
            tc.tile_pool(name="cp", bufs=1) as cp,
        ):
            psum = ps.tile([128, 64], F32)
            first = True
            for n in range(NPC):
                for q in range(4):
                    xt = xp.tile([128, QC], F32)
                    nc.sync.dma_start(xt[:], xv[:, n, q * QC : (q + 1) * QC])
                    kt = kp.tile([128, QC], F16)
                    nc.vector.tensor_scalar(
                        kt[:], xt[:], 10.0, 64.0, ALU.mult, ALU.add
                    )
                    off = (n * 4 + q) * QC
                    nc.sync.dma_start(keys[:, off : off + QC], kt[:])
                    ki = ip.tile([128, QC], I32)
                    nc.vector.tensor_copy(ki[:], kt[:])
                    nc.vector.tensor_scalar(ki[:], ki[:], 15, None, ALU.bitwise_and)

                    # interleaved planes: hi [128, QC*8], lo [128, QC*16]
                    hi = hp.tile([128, QC * 8], BF16)
                    hiv = hi[:].rearrange("p (c s) -> p s c", s=8)
                    nc.gpsimd.memset(hiv[:, 0, :], 1.0)
                    for s in range(1, 8):
                        eng = nc.gpsimd if s >= 5 else nc.vector
                        eng.tensor_scalar(
                            hiv[:, s, :], kt[:], float(16 * s), None, ALU.is_ge
                        )
                    lo = lp.tile([128, QC * 16], BF16)
                    lov = lo[:].rearrange("p (c r) -> p r c", r=16)
                    for r in range(16):
                        nc.vector.tensor_scalar(
                            lov[:, r, :], ki[:], r, None, ALU.is_equal
                        )

                    hb = hi[:].rearrange("p (b gs) -> p b gs", gs=PG * 8)
                    lb = lo[:].rearrange("p (b gr) -> p b gr", gr=PG * 16)
                    for b in range(NBLK):
                        nc.tensor.matmul(
                            psum[:],
                            lb[:, b],
                            hb[:, b],
                            start=first,
                            stop=(n == NPC - 1 and q == 3 and b == NBLK - 1),
                        )
                        first = False
            csb = cp.tile([128, 64], F32)
            nc.vector.tensor_copy(csb[:], psum[:])
            nc.sync.dma_start(counts[:], csb[:])
    nc.compile()
    return nc


def build_phase2_fast():
    """fp16 pooling with the 23-pass range-reduced select-chain sweep."""
    nc = bacc.Bacc("TRN2", target_bir_lowering=False, debug=False)
    keys = nc.dram_tensor("keys", [128, NPC * FREE], F16, kind="ExternalInput")
    values = nc.dram_tensor("values", [128, VCOLS], F32, kind="ExternalInput")
    out = nc.dram_tensor("out", [NPC, OH, OW, C], F32, kind="ExternalOutput")
    ov = out[:].rearrange("n oh ow c -> oh n (ow c)")  # [64, NPC, HO]

    with tile.TileContext(nc) as tc:
        with (
            tc.tile_pool(name="kp", bufs=2) as kp,
            tc.tile_pool(name="ap", bufs=1) as ap_,
            tc.tile_pool(name="pp", bufs=1) as pp,
            tc.tile_pool(name="ep", bufs=1) as ep,
            tc.tile_pool(name="const", bufs=1) as constp,
        ):
            vals = constp.tile([128, VCOLS], F32)
            nc.sync.dma_start(vals[:], values[:])
            rbias = vals[:, VC_RB : VC_RB + 1]

            for n in range(NPC):
                kt = kp.tile([128, FREE], F16)
                nc.sync.dma_start(kt[:], keys[:, n * FREE : (n + 1) * FREE])

                acc = ap_.tile([128, FREE], F16)
                # pass 0: init + overflow tails
                nc.vector._custom_dve(
                    RANGEC,
                    out=acc[:],
                    in0=kt[:],
                    s0=vals[:, VC_S0 : VC_S0 + 1],
                    s1=vals[:, VC_S1 : VC_S1 + 1],
                    imm2=CMID,
                )
                # LUT2 passes (max-accumulate; 0/OVF init preserved)
                for j in range(NP2):
                    sj = 1 + j
                    nc.vector._custom_dve(
                        LUT2,
                        out=acc[:],
                        in0=kt[:],
                        in1=acc[:],
                        s0=vals[:, VC_S0 + sj : VC_S0 + sj + 1],
                        s1=vals[:, VC_S1 + sj : VC_S1 + sj + 1],
                        imm2=float(P2BASE[j]),
                    )

                # comparable = S - dj - 2*di; dj: in-place -1 on odd-w cols
                av = acc[:].rearrange(
                    "p (ow dj c) -> p ow dj c", ow=OW, dj=2, c=C
                )
                kv = kt[:].rearrange(
                    "p (ow dj c) -> p ow dj c", ow=OW, dj=2, c=C
                )
                nc.vector.tensor_scalar(
                    av[:, :, 1, :], av[:, :, 1, :], 1.0, None, ALU.subtract
                )

                # dj winner: mask + max + key-select
                mk1 = pp.tile([128, HO], F16, tag="mk1")
                m1 = pp.tile([128, HO], F16, tag="m1")
                kd = pp.tile([128, HO], F16, tag="kd")
                m1v = m1[:].rearrange("p (ow c) -> p ow c", c=C)
                mk1v = mk1[:].rearrange("p (ow c) -> p ow c", c=C)
                nc.vector.tensor_tensor(
                    mk1v, av[:, :, 0, :], av[:, :, 1, :], ALU.is_ge
                )
                nc.vector.tensor_tensor(
                    m1v, av[:, :, 0, :], av[:, :, 1, :], ALU.max
                )
                # kd = kO + mask*(kE - kO)  (copy_predicated can't mix
                # strided and contiguous views in the interp)
                kdv = kd[:].rearrange("p (ow c) -> p ow c", c=C)
                nc.vector.tensor_tensor(kdv, kv[:, :, 0, :], kv[:, :, 1, :], ALU.subtract)
                nc.vector.tensor_tensor(kd[:], kd[:], mk1[:], ALU.mult)
                nc.vector.tensor_tensor(kdv, kdv, kv[:, :, 1, :], ALU.add)

                # di winner across adjacent partitions
                nc.vector.tensor_scalar(m1[:], m1[:], rbias, None, ALU.subtract)
                shm = pp.tile([128, HO], F16, tag="shm")
                nc.vector.stream_shuffle(shm[:], m1[:], [i ^ 1 for i in range(32)])
                mk2 = pp.tile([128, HO], I16, tag="mk2")
                nc.vector.tensor_tensor(mk2[:], m1[:], shm[:], ALU.is_ge)
                shk = pp.tile([128, HO], F16, tag="shk")
                nc.vector.stream_shuffle(shk[:], kd[:], [i ^ 1 for i in range(32)])
                kf = pp.tile([128, HO], F16, tag="kf")
                nc.vector.select(kf[:], mk2[:], kd[:], shk[:])

                # value = (key - 64) * 0.1
                ext = ep.tile([128, HO], F32)
                nc.vector.tensor_scalar(
                    ext[:], kf[:], -64.0, 0.1, ALU.add, ALU.mult
                )
                nc.sync.dma_start(ov[:, n, :], ext[0::2, :])
    nc.compile()
    return nc


def build_phase2():
    """Fallback: exact full-range f32 sweep (original baseline path)."""
    nc = bacc.Bacc("TRN2", target_bir_lowering=False, debug=False)
    keys = nc.dram_tensor("keys", [128, NPC * FREE], F16, kind="ExternalInput")
    values = nc.dram_tensor("values", [128, NB + 1], F32, kind="ExternalInput")
    out = nc.dram_tensor("out", [NPC, OH, OW, C], F32, kind="ExternalOutput")
    ov = out[:].rearrange("n oh ow c -> oh n (ow c)")  # [64, NPC, OW*C]

    with tile.TileContext(nc) as tc:
        with (
            tc.tile_pool(name="kp", bufs=2) as kp,
            tc.tile_pool(name="ap", bufs=1) as ap_,
            tc.tile_pool(name="pp", bufs=1) as pp,
            tc.tile_pool(name="ep", bufs=1) as ep,
            tc.tile_pool(name="const", bufs=1) as constp,
        ):
            vals = constp.tile([128, NB + 1], F32)
            nc.sync.dma_start(vals[:], values[:])
            rbias = vals[:, NB : NB + 1]  # 256*(p%2), host-provided

            # kpos[p, w, c] = 128 * (w%2)
            kpos = constp.tile([128, FREE], F32, tag="kpos")
            kv = kpos[:].rearrange("p (w c) -> p w c", c=C)
            nc.vector.memset(kv[:, 0::2, :], 0.0)
            nc.vector.memset(kv[:, 1::2, :], 128.0)

            for n in range(NPC):
                kt = kp.tile([128, FREE], F16)
                nc.sync.dma_start(kt[:], keys[:, n * FREE : (n + 1) * FREE])

                acc = ap_.tile([128, FREE], F32)
                nc.gpsimd.memset(acc[:], 0.0)
                for j in range(NSWEEP // 2):
                    b0 = BIN_LO + 2 * j
                    nc.vector._custom_dve(
                        LUT2,
                        out=acc[:],
                        in0=kt[:],
                        in1=acc[:],
                        s0=vals[:, b0 : b0 + 1],
                        s1=vals[:, b0 + 1 : b0 + 2],
                        imm2=float(b0),
                    )
                # comparable = acc - 128*dj - 256*di
                nc.vector.tensor_tensor(acc[:], acc[:], kpos[:], ALU.subtract)

                # max over dj (within partition)
                pooled = pp.tile([128, HO], F32)
                av = acc[:].rearrange("p (ow dj c) -> p ow dj c", ow=OW, dj=2, c=C)
                pj = pooled[:].rearrange("p (ow c) -> p ow c", c=C)
                nc.vector.tensor_tensor(pj, av[:, :, 0, :], av[:, :, 1, :], ALU.max)
                nc.vector.tensor_scalar(pooled[:], pooled[:], rbias, None, ALU.subtract)

                # max over di (adjacent partition pairs)
                shuf = pp.tile([128, HO], F32, tag="shuf")
                nc.vector.stream_shuffle(shuf[:], pooled[:], [i ^ 1 for i in range(32)])
                nc.vector.tensor_tensor(pooled[:], pooled[:], shuf[:], ALU.max)

                # u = BIG - m; key = u & 127; v = (key-64)/10
                ext = ep.tile([128, HO], F32)
                nc.vector.tensor_scalar(ext[:], pooled[:], -1.0, BIG, ALU.mult, ALU.add)
                exi = ep.tile([128, HO], I32, tag="exi")
                nc.vector.tensor_copy(exi[:], ext[:])
                nc.vector.tensor_scalar(exi[:], exi[:], 127, None, ALU.bitwise_and)
                nc.vector.tensor_copy(ext[:], exi[:])
                nc.vector.tensor_scalar(ext[:], ext[:], -64.0, 0.1, ALU.add, ALU.mult)

                nc.sync.dma_start(ov[:, n, :], ext[0::2, :])
    nc.compile()
    return nc


_CACHE = {}


def _get(name, builder):
    if name not in _CACHE:
        _CACHE[name] = builder()
    return _CACHE[name]


# --------------------------------------------------------------------------
# Host orchestration
# --------------------------------------------------------------------------


def _rank_table(counts: np.ndarray):
    """Replicate the reference's f32 entropy per bin and rank bins by it
    (equal f32 entropy -> equal rank). counts: int64[NB]."""
    size = np.float32(counts.sum())
    present = counts > 0
    p = counts.astype(np.float32) / size  # f32 division, like jnp
    with np.errstate(divide="ignore", invalid="ignore"):
        ent = (-p * np.log(p.astype(np.float32)).astype(np.float32)).astype(np.float32)
    ent[~present] = np.inf
    # rank by entropy ascending; equal ent values share a rank
    uniq = np.unique(ent[present])  # sorted ascending
    rank = np.zeros(NB, dtype=np.int64)
    rank[present] = np.searchsorted(uniq, ent[present])
    return rank, present, len(uniq)


def _merge_counts(res1) -> np.ndarray:
    counts = np.zeros(NB, dtype=np.int64)
    for r in res1:
        a = np.round(r["counts"].astype(np.float64)).astype(np.int64)
        a = a.reshape(8, 16, 8, 8)  # [(pg, r), (pg', s)]
        c2 = np.zeros((9, 16), dtype=np.int64)
        c2[:8] = np.einsum("grgs->sr", a)  # pg == pg' diagonal blocks
        cnt = c2[:8] - c2[1:]  # thermometer difference over s
        counts += cnt.reshape(NB)
    return counts


def _fast_vals(rank: np.ndarray, present: np.ndarray, nr: int) -> np.ndarray:
    """Assemble the vals tensor for the fast path. Score
    S[b] = (nr - rank[b])*4 + 3; OVF = 4*(nr+1)+3."""
    S = np.zeros(NB, dtype=np.float64)
    S[present] = (nr - rank[present]) * 4 + 3
    ovf = 4 * (nr + 1) + 3
    v = np.zeros(VCOLS, dtype=np.float64)
    # pass 0 (RANGEC): s0 = radius, s1 = overflow score
    v[VC_S0] = CRAD
    v[VC_S1] = ovf
    for j, b in enumerate(P2BASE):
        v[VC_S0 + 1 + j] = S[b]
        v[VC_S1 + 1 + j] = S[b + 1]
    vals = np.broadcast_to(v, (128, VCOLS)).copy()
    vals[:, VC_RB] = 2.0 * (np.arange(128) % 2)
    # fp32 tensor (custom-DVE scalar slots require f32); scores are
    # fp16-exact so fp16 acc stores are lossless.
    assert np.all(S.astype(np.float16).astype(np.float64) == S)
    assert ovf == np.float64(np.float16(ovf))
    return vals.astype(np.float32)


def kernel(inputs: np.ndarray) -> np.ndarray:
    x = np.ascontiguousarray(np.asarray(inputs, dtype=np.float32))
    assert x.shape == (N, H, W, C), x.shape

    core_ids = list(range(N_CORES))
    shards = [x[i * NPC : (i + 1) * NPC] for i in range(N_CORES)]

    # ---- phase 1: exact global histogram --------------------------------
    nc1 = _get("p1", build_phase1)
    in_maps = [{"x": s} for s in shards]
    res1 = run_bass_kernel_spmd(nc1, in_maps, core_ids).results
    keys_list = [r["keys"] for r in res1]
    counts = _merge_counts(res1)
    total = int(counts.sum())
    assert total == N * H * W * C, (
        f"histogram lost elements: {total} != {N * H * W * C} "
        "(keys outside [0,128)?)"
    )
    # entropy is strictly increasing in count only below p = 1/e
    assert counts.max() / total < 0.3678, "p_max >= 1/e; rank ordering invalid"
    assert counts[:BIN_LO].sum() == 0 and counts[BIN_LO + NSWEEP :].sum() == 0, (
        "keys outside the phase-2 sweep window"
    )

    rank, present, nr = _rank_table(counts)

    # ---- fast-path admissibility ----------------------------------------
    inr = np.zeros(NB, dtype=bool)
    inr[RLO : RHI + 1] = True
    tail_counts = counts[~inr & present]
    in_counts = counts[inr & present]
    fast_ok = (
        in_counts.size > 0
        and (tail_counts.size == 0 or tail_counts.max() < in_counts.min())
        and 4 * (nr + 1) + 3 <= 2048  # fp16-exact scores
    )

    if fast_ok:
        vals_np = _fast_vals(rank, present, nr)
        nc2 = _get("p2f", build_phase2_fast)
        in_maps2 = [{"keys": k, "values": vals_np} for k in keys_list]
    else:
        assert rank.max() * 512 + NB <= 65535
        tbl = np.zeros(NB, dtype=np.float32)
        b = np.arange(NB)
        tbl[present] = BIG - (rank[present] * 512 + b[present]).astype(np.float32)
        vals_np = np.zeros((128, NB + 1), dtype=np.float32)
        vals_np[:, :NB] = tbl
        vals_np[1::2, NB] = 256.0  # rbias: 256*(h%2) for the di tie-break
        nc2 = _get("p2", build_phase2)
        in_maps2 = [{"keys": k, "values": vals_np} for k in keys_list]

    res2 = run_bass_kernel_spmd(nc2, in_maps2, core_ids).results
    out = np.concatenate([r["out"] for r in res2], axis=0)
    assert out.shape == (N, OH, OW, C)
    return out
